# revision 75
# baseline (speedup 1.0000x reference)
"""Trainium2 Bass kernel for nn_Attn_11536282157393 (causal attention block).

Computes, for x:[2,2048,2048] f32:
    qkv = x @ W_qkv + b_qkv ; split heads (16 x 128)
    q,k = rope(rms_norm(q/k)) ; causal softmax(q k^T / sqrt(d)) @ v
    out = ctx @ W_out + b_out

Sharding over 8 NeuronCores: heads 2r,2r+1 on core r (QKV column-parallel),
x-transpose token-sharded + AllGather, output projection column-parallel
(core r computes out[:, :, 256r:256r+256]) with a ctx AllGather in between.
All matmuls run in float32r (TF32-like, ~1e-4 rel err).
"""
import sys

sys.path.insert(0, "/opt/trn_rl_repo")

from contextlib import ExitStack

import numpy as np

import concourse.bacc as bacc
import concourse.bass as bass
import concourse.mybir as mybir
import concourse.tile as tile

F32 = mybir.dt.float32
F32R = mybir.dt.float32r

B = 2
L = 2048
D = 2048
NH = 16
HD = 128  # head dim
NC = 8  # cores
HPC = NH // NC  # heads per core = 2
TOK = B * L  # 4096 global tokens
TOK_PC = TOK // NC  # 512 tokens per core for the x-transpose shard
ROPE_BASE = 10000.0
EPS = 1e-6
P = 128  # partitions
NKT = D // P  # 16 k-tiles over the model dim
NMT = TOK // P  # 32 token tiles
LQ_CHUNK = 512
NJ = L // LQ_CHUNK  # 4 q-chunks per batch sequence
OUT_COLS = D // NC  # 256 output columns per core


def _bcast(handle, n_part, n_cols):
    """AP reading a [1, n_cols] dram tensor broadcast across n_part partitions."""
    return bass.AP(tensor=handle, offset=0, ap=[[0, n_part], [1, n_cols]])


import os

NO_CC = os.environ.get("ATTN_NO_CC", "0") == "1"


def _build_program():
    nc = bacc.Bacc("TRN2", target_bir_lowering=False, debug=False, num_devices=NC)

    # ---- external I/O (per core) ----
    x_sl = nc.dram_tensor("x_slice", [TOK_PC, D], F32, kind="ExternalInput")
    w_qkv = nc.dram_tensor("w_qkv", [D, 6 * HD], F32, kind="ExternalInput")
    b_qkv = nc.dram_tensor("b_qkv", [1, 6 * HD], F32, kind="ExternalInput")
    w_out = nc.dram_tensor("w_out", [D, OUT_COLS], F32, kind="ExternalInput")
    b_out = nc.dram_tensor("b_out", [1, OUT_COLS], F32, kind="ExternalInput")
    cos_in = nc.dram_tensor("cos", [L, HD // 2], F32, kind="ExternalInput")
    sin_in = nc.dram_tensor("sin", [L, HD // 2], F32, kind="ExternalInput")
    out_sl = nc.dram_tensor("out_slice", [TOK, OUT_COLS], F32, kind="ExternalOutput")

    # ---- inline consts ----
    ident_c = nc.inline_tensor(np.eye(P, dtype=np.float32), "ident_c")
    ones_c = nc.inline_tensor(np.ones((P, 1), dtype=np.float32), "ones_c")
    # diagonal-block causal masks in scoresT layout: keep iff iq >= ik + 128*c
    iq = np.arange(LQ_CHUNK)[None, :]
    ik = np.arange(P)[:, None]
    masks_np = np.stack(
        [(iq >= ik + P * c).astype(np.float32) for c in range(4)], axis=1
    )  # [128, 4, 512]
    masks_c = nc.inline_tensor(np.ascontiguousarray(masks_np), "masks_c")

    # ---- DRAM scratch ----
    # xT is gathered in 4 chunks (one per local 128-token tile) so the
    # AllGathers pipeline with the transposes and the QKV matmuls.
    NML = TOK_PC // P  # 4 local token tiles
    xt_locals = [nc.dram_tensor(f"xt_local{i}", [D, P], F32R) for i in range(NML)]
    xt_fulls = [
        nc.dram_tensor(f"xt_full{i}", [NC, D, P], F32R, addr_space="Shared")
        for i in range(NML)
    ]
    v_dram = nc.dram_tensor("v_dram", [TOK, HPC * HD], F32R)
    # ctx is gathered per (batch, sequence-chunk j) covering both local heads:
    # the out-projection token tile m depends only on chunk j=m//4, so it
    # starts while later chunks of attention are still computing; only one
    # gather remains after the last attention chunk.
    ctx_local = nc.dram_tensor("ctx_local", [B, NJ, HPC, P, LQ_CHUNK], F32R)
    ctx_fulls = [
        [
            nc.dram_tensor(
                f"ctx_full{b}_{j}", [NC, HPC, P, LQ_CHUNK], F32R,
                addr_space="Shared",
            )
            for j in range(NJ)
        ]
        for b in range(B)
    ]

    rg = [list(range(NC))]

    with tile.TileContext(nc) as tc, ExitStack() as ctx:
        consts = ctx.enter_context(tc.tile_pool(name="consts", bufs=1))

        # ---------- consts into SBUF ----------
        ident_f = consts.tile([P, P], F32)
        nc.sync.dma_start(ident_f[:], ident_c[:])
        # resident transposed q/k: [d, head, global token]
        q_res = consts.tile([P, HPC, TOK], F32R, tag="q_res")
        k_res = consts.tile([P, HPC, TOK], F32R, tag="k_res")
        eps_t = consts.tile([P, 1], F32)
        nc.vector.memset(eps_t[:], EPS)

        # ---------- phase 1: transpose my 512-token slice of x ----------
        # loads in plain f32 on HWDGE so they don't queue behind the big
        # SWDGE weight casts; the PSUM->SBUF copy rounds to f32r
        with (
            tc.tile_pool(name="xtp", bufs=4) as xtp,
            tc.tile_pool(name="xtpp", bufs=4, space="PSUM") as xtpp,
        ):
            x_r = x_sl[:].rearrange("(m p) d -> m p d", p=P)  # [4, 128, 2048]
            for m in range(TOK_PC // P):
                xin = xtp.tile([P, NKT, P], F32, tag="xin")
                x_rm = x_r[m].rearrange("p (k q) -> p k q", q=P)
                for c in range(4):  # split across queues: 4x256KB in parallel
                    eng = nc.sync if c % 2 == 0 else nc.scalar
                    eng.dma_start(
                        xin[:, 4 * c : 4 * c + 4, :], x_rm[:, 4 * c : 4 * c + 4, :]
                    )
                xout = xtp.tile([P, NKT, P], F32R, tag="xout")
                for k in range(NKT):
                    pst = xtpp.tile([P, P], F32, tag="pst")
                    nc.tensor.transpose(pst[:], xin[:, k, :], ident_f[:])
                    nc.vector.tensor_copy(xout[:, k, :], pst[:])
                xt_l_r = xt_locals[m][:].rearrange("(k p) t -> p k t", p=P)
                for c in range(2):
                    eng = nc.sync if c == 0 else nc.scalar
                    eng.dma_start(
                        xt_l_r[:, 8 * c : 8 * c + 8, :],
                        xout[:, 8 * c : 8 * c + 8, :],
                    )
                # phase 2 (split): AllGather this token tile immediately
                if NO_CC:
                    nc.gpsimd.dma_start(xt_fulls[m][0], xt_locals[m][:])
                else:
                    nc.gpsimd.collective_compute(
                        "AllGather",
                        mybir.AluOpType.bypass,
                        replica_groups=rg,
                        ins=[xt_locals[m][:]],
                        outs=[xt_fulls[m][:]],
                    )

        # consts + weights — issued after phase 1 so their DMA traffic
        # overlaps the xT AllGathers instead of delaying the first transposes
        ident = consts.tile([P, P], F32R)
        nc.gpsimd.dma_start(ident[:], ident_c[:])
        ones_col = consts.tile([P, 1], F32R)
        nc.gpsimd.dma_start(ones_col[:], ones_c[:])
        masks = consts.tile([P, 4, LQ_CHUNK], F32)
        nc.sync.dma_start(masks[:], masks_c[:])
        # cos/sin replicated x2 in free dim for a (h0,h1) head pair rope
        cos2 = consts.tile([P, L // P, 2, HD // 2], F32)
        sin2 = consts.tile([P, L // P, 2, HD // 2], F32)
        cs_src = cos_in[:].rearrange("(t p) f -> p t f", p=P)
        sn_src = sin_in[:].rearrange("(t p) f -> p t f", p=P)
        for c in range(2):
            nc.sync.dma_start(cos2[:, :, c, :], cs_src)
            nc.sync.dma_start(sin2[:, :, c, :], sn_src)
        w_qkv_sb = consts.tile([P, NKT, 6 * HD], F32R)
        w_qkv_r = w_qkv[:].rearrange("(k p) f -> p k f", p=P)
        for c in range(4):  # 4 SWDGE queues in parallel
            nc.gpsimd.dma_start(
                w_qkv_sb[:, 4 * c : 4 * c + 4, :], w_qkv_r[:, 4 * c : 4 * c + 4, :]
            )
        bias_qkv = consts.tile([P, 6 * HD], F32)
        nc.gpsimd.dma_start(bias_qkv[:], _bcast(b_qkv, P, 6 * HD))
        bias_out = consts.tile([P, OUT_COLS], F32)
        nc.gpsimd.dma_start(bias_out[:], _bcast(b_out, P, OUT_COLS))

        # ---------- phase 3: QKV projection, rmsnorm+rope, transposes ----------
        # qkv feature order in w_qkv: [q_h0 q_h1 k_h0 k_h1 v_h0 v_h1]
        with (
            tc.tile_pool(name="qkvp", bufs=3) as qkvp,
            tc.tile_pool(name="qkv_ps", bufs=2, space="PSUM") as qkv_ps,
            tc.tile_pool(name="tr_ps", bufs=2, space="PSUM") as tr_ps,
        ):
            for mi in range(NMT):
                # m_local-major order: tile mi depends only on AllGather #ml,
                # so QKV starts as soon as the first xT chunk has gathered.
                ml, blk = mi // NC, mi % NC
                m = blk * NML + ml  # global token tile this iteration handles
                ps_qk = qkv_ps.tile([P, 4 * HD], F32, tag="ps_qk")
                ps_v = qkv_ps.tile([P, 2 * HD], F32, tag="ps_v")
                xt_m = qkvp.tile([P, NKT, P], F32R, tag="xt_m")
                xt_src = xt_fulls[ml][blk].rearrange("(k p) t -> p k t", p=P)
                for c in range(2):
                    eng = nc.sync if (m + c) % 2 == 0 else nc.scalar
                    eng.dma_start(
                        xt_m[:, 8 * c : 8 * c + 8, :],
                        xt_src[:, 8 * c : 8 * c + 8, :],
                    )
                for k in range(NKT):
                    nc.tensor.matmul(
                        ps_qk[:], xt_m[:, k, :], w_qkv_sb[:, k, : 4 * HD],
                        start=(k == 0), stop=(k == NKT - 1),
                    )
                    nc.tensor.matmul(
                        ps_v[:], xt_m[:, k, :], w_qkv_sb[:, k, 4 * HD :],
                        start=(k == 0), stop=(k == NKT - 1),
                    )
                # bias add for q,k then rms stats
                qk_b = qkvp.tile([P, 4 * HD], F32, tag="qk_b")
                nc.vector.tensor_add(qk_b[:], ps_qk[:], bias_qkv[:, : 4 * HD])
                sq = qkvp.tile([P, 4 * HD], F32, tag="sq")
                nc.scalar.square(sq[:], qk_b[:])
                ms = qkvp.tile([P, 4], F32, tag="ms")
                nc.vector.reduce_sum(
                    out=ms[:],
                    in_=sq[:].rearrange("p (s d) -> p s d", d=HD),
                    axis=mybir.AxisListType.X,
                )
                rms = qkvp.tile([P, 4], F32, tag="rms")
                nc.scalar.activation(
                    out=rms[:], in_=ms[:], func=mybir.ActivationFunctionType.Sqrt,
                    bias=eps_t[:], scale=1.0 / HD,
                )
                rinv = qkvp.tile([P, 4], F32, tag="rinv")
                nc.vector.reciprocal(rinv[:], rms[:])
                # normalize each of the 4 slices
                qk_n = qkvp.tile([P, 4, HD], F32, tag="qk_n")
                for s in range(4):
                    nc.vector.tensor_scalar_mul(
                        qk_n[:, s, :],
                        qk_b[:, s * HD : (s + 1) * HD],
                        rinv[:, s : s + 1],
                    )
                # rope, per (q, k) head-pair
                ti = m % (L // P)
                ct = cos2[:, ti]
                st = sin2[:, ti]
                rope = qkvp.tile([P, 4, HD], F32R, tag="rope")
                for g in range(2):  # 0: q pair, 1: k pair
                    x1 = qk_n[:, 2 * g : 2 * g + 2, : HD // 2]
                    x2 = qk_n[:, 2 * g : 2 * g + 2, HD // 2 :]
                    t_a = qkvp.tile([P, 2, HD // 2], F32, tag="t_a")
                    t_b = qkvp.tile([P, 2, HD // 2], F32, tag="t_b")
                    nc.vector.tensor_mul(t_a[:], x1, ct)
                    nc.gpsimd.tensor_mul(t_b[:], x2, st)
                    nc.vector.tensor_sub(
                        rope[:, 2 * g : 2 * g + 2, : HD // 2], t_a[:], t_b[:]
                    )
                    t_c = qkvp.tile([P, 2, HD // 2], F32, tag="t_c")
                    t_d = qkvp.tile([P, 2, HD // 2], F32, tag="t_d")
                    nc.gpsimd.tensor_mul(t_c[:], x2, ct)
                    nc.vector.tensor_mul(t_d[:], x1, st)
                    nc.vector.tensor_add(
                        rope[:, 2 * g : 2 * g + 2, HD // 2 :], t_c[:], t_d[:]
                    )
                # transpose the 4 slices straight into the resident q/k bufs
                for s in range(4):
                    pst = tr_ps.tile([P, P], F32R, tag="tr")
                    nc.tensor.transpose(pst[:], rope[:, s, :], ident[:])
                    dst = q_res if s < 2 else k_res
                    nc.vector.tensor_copy(
                        dst[:, s % 2, m * P : (m + 1) * P], pst[:]
                    )
                # v: bias + copy out as f32r
                v_sb = qkvp.tile([P, 2 * HD], F32R, tag="v_sb")
                nc.vector.tensor_add(v_sb[:], ps_v[:], bias_qkv[:, 4 * HD :])
                nc.sync.dma_start(v_dram[m * P : (m + 1) * P, :], v_sb[:])

        # ---------- phase 5: attention per (b, h) ----------
        scale = 1.0 / float(np.sqrt(HD))
        with (
            tc.tile_pool(name="attp", bufs=2) as attp,
            tc.tile_pool(name="att_sm", bufs=3) as att_sm,
            tc.tile_pool(name="att_ps", bufs=2, space="PSUM") as att_ps,
        ):
            for b in range(B):
                v_sbs = []
                for h in range(HPC):
                    v_sb = attp.tile([P, L // P, HD], F32R, tag=f"v_att{h}")
                    v_src = v_dram[
                        b * L : (b + 1) * L, h * HD : (h + 1) * HD
                    ].rearrange("(t p) d -> p t d", p=P)
                    for c in range(2):
                        eng = nc.sync if c == 0 else nc.scalar
                        eng.dma_start(
                            v_sb[:, 8 * c : 8 * c + 8, :],
                            v_src[:, 8 * c : 8 * c + 8, :],
                        )
                    v_sbs.append(v_sb)
                for j in range(NJ):
                    nkt_j = 4 * (j + 1)  # causal: k-tiles 0..4j+3
                    for h in range(HPC):
                        kt_sb = k_res[:, h, b * L : (b + 1) * L]
                        qt_j = q_res[
                            :, h, b * L + j * LQ_CHUNK : b * L + (j + 1) * LQ_CHUNK
                        ]
                        v_sb = v_sbs[h]
                        ps_ctx = att_ps.tile(
                            [P, LQ_CHUNK], F32, tag="ps_ctx", bufs=2
                        )
                        ps_den = att_ps.tile(
                            [1, LQ_CHUNK], F32, tag="ps_den", bufs=2
                        )
                        for t in range(nkt_j):
                            ps_s = att_ps.tile(
                                [P, LQ_CHUNK], F32, tag="ps_s", bufs=4
                            )
                            nc.tensor.matmul(
                                ps_s[:],
                                kt_sb[:, t * P : (t + 1) * P],
                                qt_j,
                                start=True, stop=True,
                            )
                            at = att_sm.tile([P, LQ_CHUNK], F32R, tag="at", bufs=6)
                            nc.scalar.activation(
                                out=at[:], in_=ps_s[:],
                                func=mybir.ActivationFunctionType.Exp, scale=scale,
                            )
                            c = t - 4 * j
                            if c >= 0:
                                # gpsimd: DVE is the busier engine here
                                nc.gpsimd.tensor_mul(at[:], at[:], masks[:, c, :])
                            nc.tensor.matmul(
                                ps_ctx[:], v_sb[:, t, :], at[:],
                                start=(t == 0), stop=(t == nkt_j - 1),
                            )
                            nc.tensor.matmul(
                                ps_den[:], ones_col[:], at[:],
                                start=(t == 0), stop=(t == nkt_j - 1),
                            )
                        den_r = att_sm.tile([1, LQ_CHUNK], F32, tag="den_r")
                        nc.vector.reciprocal(den_r[:], ps_den[:])
                        den_b = att_sm.tile([P, LQ_CHUNK], F32, tag="den_b")
                        nc.gpsimd.partition_broadcast(den_b[:], den_r[:])
                        ctx_sb = att_sm.tile([P, LQ_CHUNK], F32R, tag="ctx_sb")
                        nc.vector.tensor_mul(ctx_sb[:], ps_ctx[:], den_b[:])
                        nc.sync.dma_start(ctx_local[b, j, h], ctx_sb[:])
                    # phase 6 (split): AllGather this (batch, chunk) for both
                    # heads as soon as the second head finishes it
                    if NO_CC:
                        nc.gpsimd.dma_start(ctx_fulls[b][j][0], ctx_local[b, j])
                    else:
                        nc.gpsimd.collective_compute(
                            "AllGather",
                            mybir.AluOpType.bypass,
                            replica_groups=rg,
                            ins=[ctx_local[b, j]],
                            outs=[ctx_fulls[b][j][:]],
                        )

        # ---------- phase 7: output projection ----------
        with (
            tc.tile_pool(name="outp", bufs=4) as outp,
            tc.tile_pool(name="outw", bufs=1) as outw,
            tc.tile_pool(name="out_ps", bufs=2, space="PSUM") as out_ps,
        ):
            w_out_sb = outw.tile([P, NKT, OUT_COLS], F32R)
            nc.gpsimd.dma_start(
                w_out_sb[:], w_out[:].rearrange("(k p) f -> p k f", p=P)
            )
            for b in range(B):
                for m in range(L // P):
                    ps_o = out_ps.tile([P, OUT_COLS], F32, tag="ps_o")
                    j, moff = m // (LQ_CHUNK // P), (m % (LQ_CHUNK // P)) * P
                    ct_t = outp.tile([P, NKT, P], F32R, tag="ct_t")
                    ct_src = ctx_fulls[b][j][:, :, :, moff : moff + P].rearrange(
                        "r h p t -> p (r h) t"
                    )
                    for c in range(2):
                        eng = nc.sync if (m + c) % 2 == 0 else nc.scalar
                        eng.dma_start(
                            ct_t[:, 8 * c : 8 * c + 8, :],
                            ct_src[:, 8 * c : 8 * c + 8, :],
                        )
                    for k in range(NKT):
                        nc.tensor.matmul(
                            ps_o[:], ct_t[:, k, :], w_out_sb[:, k, :],
                            start=(k == 0), stop=(k == NKT - 1),
                        )
                    o_sb = outp.tile([P, OUT_COLS], F32, tag="o_sb")
                    nc.vector.tensor_add(o_sb[:], ps_o[:], bias_out[:])
                    nc.sync.dma_start(
                        out_sl[(b * L + m * P) : (b * L + (m + 1) * P), :], o_sb[:]
                    )

    nc.compile()
    return nc


_PROGRAM_CACHE = {}


def _get_program():
    if "nc" not in _PROGRAM_CACHE:
        _PROGRAM_CACHE["nc"] = _build_program()
    return _PROGRAM_CACHE["nc"]


def _build_sharded_runner(nc, n_cores):
    """Like bass2jax.run_bass_via_pjrt, but jits once and is reusable."""
    import jax
    from jax.sharding import Mesh, PartitionSpec
    from jax.experimental.shard_map import shard_map
    from concourse.bass2jax import (
        _bass_exec_p,
        install_neuronx_cc_hook,
        partition_id_tensor,
    )

    install_neuronx_cc_hook()
    partition_name = nc.partition_id_tensor.name if nc.partition_id_tensor else None
    in_names, out_names, out_avals, zero_outs = [], [], [], []
    for alloc in nc.m.functions[0].allocations:
        if not isinstance(alloc, mybir.MemoryLocationSet):
            continue
        name = alloc.memorylocations[0].name
        if alloc.kind == "ExternalInput":
            if name != partition_name:
                in_names.append(name)
        elif alloc.kind == "ExternalOutput":
            out_names.append(name)
            shape = tuple(alloc.tensor_shape)
            dtype = mybir.dt.np(alloc.dtype)
            out_avals.append(jax.core.ShapedArray(shape, dtype))
            zero_outs.append(np.zeros(shape, dtype))
    n_params = len(in_names)
    n_outs = len(out_avals)
    all_names = list(in_names) + list(out_names)
    if partition_name is not None:
        all_names.append(partition_name)
    donate = tuple(range(n_params, n_params + n_outs))

    def _body(*args):
        operands = list(args)
        if partition_name is not None:
            operands.append(partition_id_tensor())
        outs = _bass_exec_p.bind(
            *operands,
            out_avals=tuple(out_avals),
            in_names=tuple(all_names),
            out_names=tuple(out_names),
            lowering_input_output_aliases=(),
            sim_require_finite=True,
            sim_require_nnan=True,
            nc=nc,
        )
        return tuple(outs)

    devices = jax.devices()[:n_cores]
    mesh = Mesh(np.asarray(devices), ("core",))
    in_specs = (PartitionSpec("core"),) * (n_params + n_outs)
    out_specs = (PartitionSpec("core"),) * n_outs
    sharded = jax.jit(
        shard_map(
            _body, mesh=mesh, in_specs=in_specs, out_specs=out_specs, check_rep=False
        ),
        donate_argnums=donate,
        keep_unused=True,
    )

    def run(in_maps):
        per_core = [[np.asarray(m[name]) for name in in_names] for m in in_maps]
        concat_in = [
            np.concatenate([per_core[c][i] for c in range(n_cores)], axis=0)
            for i in range(n_params)
        ]
        zeros = [
            np.zeros((n_cores * z.shape[0], *z.shape[1:]), z.dtype) for z in zero_outs
        ]
        outs = sharded(*concat_in, *zeros)
        return [
            {
                name: np.asarray(outs[i]).reshape(n_cores, *out_avals[i].shape)[c]
                for i, name in enumerate(out_names)
            }
            for c in range(n_cores)
        ]

    return run


def _get_runner():
    if "run" not in _PROGRAM_CACHE:
        _PROGRAM_CACHE["run"] = _build_sharded_runner(_get_program(), NC)
    return _PROGRAM_CACHE["run"]


def _host_tables():
    half = HD // 2
    inv_freq = 1.0 / (ROPE_BASE ** (np.arange(half, dtype=np.float32) / half))
    pos = np.arange(L, dtype=np.float32)
    ang = pos[:, None] * inv_freq[None, :].astype(np.float32)
    return np.cos(ang).astype(np.float32), np.sin(ang).astype(np.float32)


def make_in_maps(x, W_qkv, b_qkv, W_out, b_out):
    x2 = np.ascontiguousarray(np.asarray(x, dtype=np.float32).reshape(TOK, D))
    W_qkv = np.asarray(W_qkv, dtype=np.float32)
    b_qkv = np.asarray(b_qkv, dtype=np.float32)
    W_out = np.asarray(W_out, dtype=np.float32)
    b_out = np.asarray(b_out, dtype=np.float32)
    cos_t, sin_t = _host_tables()

    in_maps = []
    for r in range(NC):
        # feature order per core: [q_h0 q_h1 k_h0 k_h1 v_h0 v_h1], h0=2r, h1=2r+1
        cols = []
        for qkv_i in (0, 1, 2):
            for h in (2 * r, 2 * r + 1):
                c0 = qkv_i * D + h * HD
                cols.append(np.arange(c0, c0 + HD))
        cols = np.concatenate(cols)
        in_maps.append(
            {
                "x_slice": np.ascontiguousarray(x2[r * TOK_PC : (r + 1) * TOK_PC]),
                "w_qkv": np.ascontiguousarray(W_qkv[:, cols]),
                "b_qkv": np.ascontiguousarray(b_qkv[cols][None, :]),
                "w_out": np.ascontiguousarray(
                    W_out[:, r * OUT_COLS : (r + 1) * OUT_COLS]
                ),
                "b_out": np.ascontiguousarray(
                    b_out[r * OUT_COLS : (r + 1) * OUT_COLS][None, :]
                ),
                "cos": cos_t,
                "sin": sin_t,
            }
        )
    return in_maps


def kernel(x, mask, W_qkv, b_qkv, W_out, b_out):
    run = _get_runner()
    in_maps = make_in_maps(x, W_qkv, b_qkv, W_out, b_out)
    results = run(in_maps)
    parts = [results[r]["out_slice"] for r in range(NC)]
    out = np.concatenate(parts, axis=1).reshape(B, L, D)
    return np.ascontiguousarray(out.astype(np.float32))


# revision 77
# speedup vs baseline: 1.1479x; 1.1479x over previous
"""Trainium2 Bass kernel for nn_Attn_11536282157393 (causal attention block).

Computes, for x:[2,2048,2048] f32:
    qkv = x @ W_qkv + b_qkv ; split heads (16 x 128)
    q,k = rope(rms_norm(q/k)) ; causal softmax(q k^T / sqrt(d)) @ v
    out = ctx @ W_out + b_out

Sharding over 8 NeuronCores: heads 2r,2r+1 on core r (QKV column-parallel),
x-transpose token-sharded + AllGather, output projection column-parallel
(core r computes out[:, :, 256r:256r+256]) with a ctx AllGather in between.
All matmuls run in float32r (TF32-like, ~1e-4 rel err).
"""
import sys

sys.path.insert(0, "/opt/trn_rl_repo")

from contextlib import ExitStack

import numpy as np

import concourse.bacc as bacc
import concourse.bass as bass
import concourse.mybir as mybir
import concourse.tile as tile

F32 = mybir.dt.float32
F32R = mybir.dt.float32r

B = 2
L = 2048
D = 2048
NH = 16
HD = 128  # head dim
NC = 8  # cores
HPC = NH // NC  # heads per core = 2
TOK = B * L  # 4096 global tokens
TOK_PC = TOK // NC  # 512 tokens per core for the x-transpose shard
ROPE_BASE = 10000.0
EPS = 1e-6
P = 128  # partitions
NKT = D // P  # 16 k-tiles over the model dim
NMT = TOK // P  # 32 token tiles
LQ_CHUNK = 512
NJ = L // LQ_CHUNK  # 4 q-chunks per batch sequence
OUT_COLS = D // NC  # 256 output columns per core


def _bcast(handle, n_part, n_cols):
    """AP reading a [1, n_cols] dram tensor broadcast across n_part partitions."""
    return bass.AP(tensor=handle, offset=0, ap=[[0, n_part], [1, n_cols]])


import os

NO_CC = os.environ.get("ATTN_NO_CC", "0") == "1"


def _build_program():
    nc = bacc.Bacc("TRN2", target_bir_lowering=False, debug=False, num_devices=NC)

    # ---- external I/O (per core) ----
    x_sl = nc.dram_tensor("x_slice", [TOK_PC, D], F32, kind="ExternalInput")
    w_qkv = nc.dram_tensor("w_qkv", [D, 6 * HD], F32, kind="ExternalInput")
    b_qkv = nc.dram_tensor("b_qkv", [1, 6 * HD], F32, kind="ExternalInput")
    w_out = nc.dram_tensor("w_out", [D, OUT_COLS], F32, kind="ExternalInput")
    b_out = nc.dram_tensor("b_out", [1, OUT_COLS], F32, kind="ExternalInput")
    cos_in = nc.dram_tensor("cos", [L, HD // 2], F32, kind="ExternalInput")
    sin_in = nc.dram_tensor("sin", [L, HD // 2], F32, kind="ExternalInput")
    out_sl = nc.dram_tensor("out_slice", [TOK, OUT_COLS], F32, kind="ExternalOutput")

    # ---- inline consts ----
    ident_c = nc.inline_tensor(np.eye(P, dtype=np.float32), "ident_c")
    ones_c = nc.inline_tensor(np.ones((P, 1), dtype=np.float32), "ones_c")
    # diagonal-block causal masks in scoresT layout: keep iff iq >= ik + 128*c
    iq = np.arange(LQ_CHUNK)[None, :]
    ik = np.arange(P)[:, None]
    masks_np = np.stack(
        [(iq >= ik + P * c).astype(np.float32) for c in range(4)], axis=1
    )  # [128, 4, 512]
    masks_c = nc.inline_tensor(np.ascontiguousarray(masks_np), "masks_c")

    # ---- DRAM scratch ----
    # xT is gathered in 4 chunks (one per local 128-token tile) so the
    # AllGathers pipeline with the transposes and the QKV matmuls.
    NML = TOK_PC // P  # 4 local token tiles
    xt_locals = [nc.dram_tensor(f"xt_local{i}", [D, P], F32R) for i in range(NML)]
    xt_fulls = [
        nc.dram_tensor(f"xt_full{i}", [NC, D, P], F32R, addr_space="Shared")
        for i in range(NML)
    ]
    v_dram = nc.dram_tensor("v_dram", [TOK, HPC * HD], F32R)
    # ctx is gathered per (batch, sequence-chunk j) covering both local heads:
    # the out-projection token tile m depends only on chunk j=m//4, so it
    # starts while later chunks of attention are still computing; only one
    # gather remains after the last attention chunk.
    ctx_local = nc.dram_tensor("ctx_local", [B, NJ, HPC, P, LQ_CHUNK], F32R)
    ctx_fulls = [
        [
            nc.dram_tensor(
                f"ctx_full{b}_{j}", [NC, HPC, P, LQ_CHUNK], F32R,
                addr_space="Shared",
            )
            for j in range(NJ)
        ]
        for b in range(B)
    ]

    rg = [list(range(NC))]

    with tile.TileContext(nc) as tc, ExitStack() as ctx:
        consts = ctx.enter_context(tc.tile_pool(name="consts", bufs=1))

        # ---------- consts into SBUF ----------
        ident_f = consts.tile([P, P], F32)
        nc.sync.dma_start(ident_f[:], ident_c[:])
        # resident transposed q/k: [d, head, global token]
        q_res = consts.tile([P, HPC, TOK], F32R, tag="q_res")
        k_res = consts.tile([P, HPC, TOK], F32R, tag="k_res")
        eps_t = consts.tile([P, 1], F32)
        nc.vector.memset(eps_t[:], EPS)

        # ---------- phase 1: transpose my 512-token slice of x ----------
        # loads in plain f32 on HWDGE so they don't queue behind the big
        # SWDGE weight casts; the PSUM->SBUF copy rounds to f32r
        with (
            tc.tile_pool(name="xtp", bufs=4) as xtp,
            tc.tile_pool(name="xtpp", bufs=4, space="PSUM") as xtpp,
        ):
            x_r = x_sl[:].rearrange("(m p) d -> m p d", p=P)  # [4, 128, 2048]
            for m in range(TOK_PC // P):
                xin = xtp.tile([P, NKT, P], F32, tag="xin")
                x_rm = x_r[m].rearrange("p (k q) -> p k q", q=P)
                for c in range(4):  # split across queues: 4x256KB in parallel
                    eng = nc.sync if c % 2 == 0 else nc.scalar
                    eng.dma_start(
                        xin[:, 4 * c : 4 * c + 4, :], x_rm[:, 4 * c : 4 * c + 4, :]
                    )
                xout = xtp.tile([P, NKT, P], F32R, tag="xout")
                for k in range(NKT):
                    pst = xtpp.tile([P, P], F32, tag="pst")
                    nc.tensor.transpose(pst[:], xin[:, k, :], ident_f[:])
                    nc.vector.tensor_copy(xout[:, k, :], pst[:])
                xt_l_r = xt_locals[m][:].rearrange("(k p) t -> p k t", p=P)
                for c in range(2):
                    eng = nc.sync if c == 0 else nc.scalar
                    eng.dma_start(
                        xt_l_r[:, 8 * c : 8 * c + 8, :],
                        xout[:, 8 * c : 8 * c + 8, :],
                    )
                # phase 2 (split): AllGather this token tile immediately
                if NO_CC:
                    nc.gpsimd.dma_start(xt_fulls[m][0], xt_locals[m][:])
                else:
                    nc.gpsimd.collective_compute(
                        "AllGather",
                        mybir.AluOpType.bypass,
                        replica_groups=rg,
                        ins=[xt_locals[m][:]],
                        outs=[xt_fulls[m][:]],
                    )

        # consts + weights — issued after phase 1 so their DMA traffic
        # overlaps the xT AllGathers instead of delaying the first transposes
        ident = consts.tile([P, P], F32R)
        nc.gpsimd.dma_start(ident[:], ident_c[:])
        ones_col = consts.tile([P, 1], F32R)
        nc.gpsimd.dma_start(ones_col[:], ones_c[:])
        masks = consts.tile([P, 4, LQ_CHUNK], F32)
        nc.sync.dma_start(masks[:], masks_c[:])
        # cos/sin replicated x2 in free dim for a (h0,h1) head pair rope
        cos2 = consts.tile([P, L // P, 2, HD // 2], F32)
        sin2 = consts.tile([P, L // P, 2, HD // 2], F32)
        cs_src = cos_in[:].rearrange("(t p) f -> p t f", p=P)
        sn_src = sin_in[:].rearrange("(t p) f -> p t f", p=P)
        for c in range(2):
            nc.sync.dma_start(cos2[:, :, c, :], cs_src)
            nc.sync.dma_start(sin2[:, :, c, :], sn_src)
        w_qkv_sb = consts.tile([P, NKT, 6 * HD], F32R)
        w_qkv_r = w_qkv[:].rearrange("(k p) f -> p k f", p=P)
        for c in range(4):  # 4 SWDGE queues in parallel
            nc.gpsimd.dma_start(
                w_qkv_sb[:, 4 * c : 4 * c + 4, :], w_qkv_r[:, 4 * c : 4 * c + 4, :]
            )
        bias_qkv = consts.tile([P, 6 * HD], F32)
        nc.gpsimd.dma_start(bias_qkv[:], _bcast(b_qkv, P, 6 * HD))
        bias_out = consts.tile([P, OUT_COLS], F32)
        nc.gpsimd.dma_start(bias_out[:], _bcast(b_out, P, OUT_COLS))

        # ---------- phase 3: QKV projection, rmsnorm+rope, transposes ----------
        # qkv feature order in w_qkv: [q_h0 q_h1 k_h0 k_h1 v_h0 v_h1]
        with (
            tc.tile_pool(name="qkvp", bufs=3) as qkvp,
            tc.tile_pool(name="qkv_ps", bufs=2, space="PSUM") as qkv_ps,
            tc.tile_pool(name="tr_ps", bufs=2, space="PSUM") as tr_ps,
        ):
            for mi in range(NMT):
                # m_local-major order: tile mi depends only on AllGather #ml,
                # so QKV starts as soon as the first xT chunk has gathered.
                ml, blk = mi // NC, mi % NC
                m = blk * NML + ml  # global token tile this iteration handles
                ps_qk = qkv_ps.tile([P, 4 * HD], F32, tag="ps_qk")
                ps_v = qkv_ps.tile([P, 2 * HD], F32, tag="ps_v")
                xt_m = qkvp.tile([P, NKT, P], F32R, tag="xt_m")
                xt_src = xt_fulls[ml][blk].rearrange("(k p) t -> p k t", p=P)
                for c in range(2):
                    eng = nc.sync if (m + c) % 2 == 0 else nc.scalar
                    eng.dma_start(
                        xt_m[:, 8 * c : 8 * c + 8, :],
                        xt_src[:, 8 * c : 8 * c + 8, :],
                    )
                for k in range(NKT):
                    nc.tensor.matmul(
                        ps_qk[:], xt_m[:, k, :], w_qkv_sb[:, k, : 4 * HD],
                        start=(k == 0), stop=(k == NKT - 1),
                    )
                    nc.tensor.matmul(
                        ps_v[:], xt_m[:, k, :], w_qkv_sb[:, k, 4 * HD :],
                        start=(k == 0), stop=(k == NKT - 1),
                    )
                # bias add for q,k then rms stats
                qk_b = qkvp.tile([P, 4 * HD], F32, tag="qk_b")
                nc.vector.tensor_add(qk_b[:], ps_qk[:], bias_qkv[:, : 4 * HD])
                sq = qkvp.tile([P, 4 * HD], F32, tag="sq")
                nc.scalar.square(sq[:], qk_b[:])
                ms = qkvp.tile([P, 4], F32, tag="ms")
                nc.vector.reduce_sum(
                    out=ms[:],
                    in_=sq[:].rearrange("p (s d) -> p s d", d=HD),
                    axis=mybir.AxisListType.X,
                )
                rms = qkvp.tile([P, 4], F32, tag="rms")
                nc.scalar.activation(
                    out=rms[:], in_=ms[:], func=mybir.ActivationFunctionType.Sqrt,
                    bias=eps_t[:], scale=1.0 / HD,
                )
                rinv = qkvp.tile([P, 4], F32, tag="rinv")
                nc.vector.reciprocal(rinv[:], rms[:])
                # normalize each of the 4 slices
                qk_n = qkvp.tile([P, 4, HD], F32, tag="qk_n")
                for s in range(4):
                    nc.vector.tensor_scalar_mul(
                        qk_n[:, s, :],
                        qk_b[:, s * HD : (s + 1) * HD],
                        rinv[:, s : s + 1],
                    )
                # rope, per (q, k) head-pair
                ti = m % (L // P)
                ct = cos2[:, ti]
                st = sin2[:, ti]
                rope = qkvp.tile([P, 4, HD], F32R, tag="rope")
                for g in range(2):  # 0: q pair, 1: k pair
                    x1 = qk_n[:, 2 * g : 2 * g + 2, : HD // 2]
                    x2 = qk_n[:, 2 * g : 2 * g + 2, HD // 2 :]
                    t_a = qkvp.tile([P, 2, HD // 2], F32, tag="t_a")
                    t_b = qkvp.tile([P, 2, HD // 2], F32, tag="t_b")
                    nc.vector.tensor_mul(t_a[:], x1, ct)
                    nc.gpsimd.tensor_mul(t_b[:], x2, st)
                    nc.vector.tensor_sub(
                        rope[:, 2 * g : 2 * g + 2, : HD // 2], t_a[:], t_b[:]
                    )
                    t_c = qkvp.tile([P, 2, HD // 2], F32, tag="t_c")
                    t_d = qkvp.tile([P, 2, HD // 2], F32, tag="t_d")
                    nc.gpsimd.tensor_mul(t_c[:], x2, ct)
                    nc.vector.tensor_mul(t_d[:], x1, st)
                    nc.vector.tensor_add(
                        rope[:, 2 * g : 2 * g + 2, HD // 2 :], t_c[:], t_d[:]
                    )
                # transpose the 4 slices straight into the resident q/k bufs
                for s in range(4):
                    pst = tr_ps.tile([P, P], F32R, tag="tr")
                    nc.tensor.transpose(pst[:], rope[:, s, :], ident[:])
                    dst = q_res if s < 2 else k_res
                    nc.vector.tensor_copy(
                        dst[:, s % 2, m * P : (m + 1) * P], pst[:]
                    )
                # v: bias + copy out as f32r
                v_sb = qkvp.tile([P, 2 * HD], F32R, tag="v_sb")
                nc.vector.tensor_add(v_sb[:], ps_v[:], bias_qkv[:, 4 * HD :])
                nc.sync.dma_start(v_dram[m * P : (m + 1) * P, :], v_sb[:])

        # ---------- phase 5: attention per (b, h) ----------
        scale = 1.0 / float(np.sqrt(HD))
        with (
            tc.tile_pool(name="attp", bufs=2) as attp,
            tc.tile_pool(name="att_sm", bufs=3) as att_sm,
            tc.tile_pool(name="att_ps", bufs=2, space="PSUM") as att_ps,
        ):
            for b in range(B):
                v_sbs = []
                for h in range(HPC):
                    v_sb = attp.tile([P, L // P, HD], F32R, tag=f"v_att{h}")
                    v_src = v_dram[
                        b * L : (b + 1) * L, h * HD : (h + 1) * HD
                    ].rearrange("(t p) d -> p t d", p=P)
                    for c in range(2):
                        eng = nc.sync if c == 0 else nc.scalar
                        eng.dma_start(
                            v_sb[:, 8 * c : 8 * c + 8, :],
                            v_src[:, 8 * c : 8 * c + 8, :],
                        )
                    v_sbs.append(v_sb)
                for j in range(NJ):
                    nkt_j = 4 * (j + 1)  # causal: k-tiles 0..4j+3
                    for h in range(HPC):
                        kt_sb = k_res[:, h, b * L : (b + 1) * L]
                        qt_j = q_res[
                            :, h, b * L + j * LQ_CHUNK : b * L + (j + 1) * LQ_CHUNK
                        ]
                        v_sb = v_sbs[h]
                        ps_ctx = att_ps.tile(
                            [P, LQ_CHUNK], F32, tag="ps_ctx", bufs=2
                        )
                        ps_den = att_ps.tile(
                            [1, LQ_CHUNK], F32, tag="ps_den", bufs=2
                        )
                        for t in range(nkt_j):
                            ps_s = att_ps.tile(
                                [P, LQ_CHUNK], F32, tag="ps_s", bufs=4
                            )
                            nc.tensor.matmul(
                                ps_s[:],
                                kt_sb[:, t * P : (t + 1) * P],
                                qt_j,
                                start=True, stop=True,
                            )
                            at = att_sm.tile([P, LQ_CHUNK], F32R, tag="at", bufs=6)
                            nc.scalar.activation(
                                out=at[:], in_=ps_s[:],
                                func=mybir.ActivationFunctionType.Exp, scale=scale,
                            )
                            c = t - 4 * j
                            if c >= 0:
                                # gpsimd: DVE is the busier engine here
                                nc.gpsimd.tensor_mul(at[:], at[:], masks[:, c, :])
                            nc.tensor.matmul(
                                ps_ctx[:], v_sb[:, t, :], at[:],
                                start=(t == 0), stop=(t == nkt_j - 1),
                            )
                            nc.tensor.matmul(
                                ps_den[:], ones_col[:], at[:],
                                start=(t == 0), stop=(t == nkt_j - 1),
                            )
                        den_r = att_sm.tile([1, LQ_CHUNK], F32, tag="den_r")
                        nc.vector.reciprocal(den_r[:], ps_den[:])
                        den_b = att_sm.tile([P, LQ_CHUNK], F32, tag="den_b")
                        nc.gpsimd.partition_broadcast(den_b[:], den_r[:])
                        ctx_sb = att_sm.tile([P, LQ_CHUNK], F32R, tag="ctx_sb")
                        nc.vector.tensor_mul(ctx_sb[:], ps_ctx[:], den_b[:])
                        nc.sync.dma_start(ctx_local[b, j, h], ctx_sb[:])
                    # phase 6 (split): AllGather this (batch, chunk) for both
                    # heads as soon as the second head finishes it
                    if NO_CC:
                        nc.gpsimd.dma_start(ctx_fulls[b][j][0], ctx_local[b, j])
                    else:
                        nc.gpsimd.collective_compute(
                            "AllGather",
                            mybir.AluOpType.bypass,
                            replica_groups=rg,
                            ins=[ctx_local[b, j]],
                            outs=[ctx_fulls[b][j][:]],
                        )

        # ---------- phase 7: output projection ----------
        with (
            tc.tile_pool(name="outp", bufs=4) as outp,
            tc.tile_pool(name="outw", bufs=1) as outw,
            tc.tile_pool(name="out_ps", bufs=2, space="PSUM") as out_ps,
        ):
            w_out_sb = outw.tile([P, NKT, OUT_COLS], F32R)
            nc.gpsimd.dma_start(
                w_out_sb[:], w_out[:].rearrange("(k p) f -> p k f", p=P)
            )
            for b in range(B):
                for m in range(L // P):
                    ps_o = out_ps.tile([P, OUT_COLS], F32, tag="ps_o")
                    j, moff = m // (LQ_CHUNK // P), (m % (LQ_CHUNK // P)) * P
                    ct_t = outp.tile([P, NKT, P], F32R, tag="ct_t", bufs=4)
                    ct_src = ctx_fulls[b][j][:, :, :, moff : moff + P].rearrange(
                        "r h p t -> p (r h) t"
                    )
                    for c in range(4):
                        eng = nc.sync if (m + c) % 2 == 0 else nc.scalar
                        eng.dma_start(
                            ct_t[:, 4 * c : 4 * c + 4, :],
                            ct_src[:, 4 * c : 4 * c + 4, :],
                        )
                    for k in range(NKT):
                        nc.tensor.matmul(
                            ps_o[:], ct_t[:, k, :], w_out_sb[:, k, :],
                            start=(k == 0), stop=(k == NKT - 1),
                        )
                    o_sb = outp.tile([P, OUT_COLS], F32, tag="o_sb")
                    nc.vector.tensor_add(o_sb[:], ps_o[:], bias_out[:])
                    nc.sync.dma_start(
                        out_sl[(b * L + m * P) : (b * L + (m + 1) * P), :], o_sb[:]
                    )

    nc.compile()
    return nc


_PROGRAM_CACHE = {}


def _get_program():
    if "nc" not in _PROGRAM_CACHE:
        _PROGRAM_CACHE["nc"] = _build_program()
    return _PROGRAM_CACHE["nc"]


def _build_sharded_runner(nc, n_cores):
    """Like bass2jax.run_bass_via_pjrt, but jits once and is reusable."""
    import jax
    from jax.sharding import Mesh, PartitionSpec
    from jax.experimental.shard_map import shard_map
    from concourse.bass2jax import (
        _bass_exec_p,
        install_neuronx_cc_hook,
        partition_id_tensor,
    )

    install_neuronx_cc_hook()
    partition_name = nc.partition_id_tensor.name if nc.partition_id_tensor else None
    in_names, out_names, out_avals, zero_outs = [], [], [], []
    for alloc in nc.m.functions[0].allocations:
        if not isinstance(alloc, mybir.MemoryLocationSet):
            continue
        name = alloc.memorylocations[0].name
        if alloc.kind == "ExternalInput":
            if name != partition_name:
                in_names.append(name)
        elif alloc.kind == "ExternalOutput":
            out_names.append(name)
            shape = tuple(alloc.tensor_shape)
            dtype = mybir.dt.np(alloc.dtype)
            out_avals.append(jax.core.ShapedArray(shape, dtype))
            zero_outs.append(np.zeros(shape, dtype))
    n_params = len(in_names)
    n_outs = len(out_avals)
    all_names = list(in_names) + list(out_names)
    if partition_name is not None:
        all_names.append(partition_name)
    donate = tuple(range(n_params, n_params + n_outs))

    def _body(*args):
        operands = list(args)
        if partition_name is not None:
            operands.append(partition_id_tensor())
        outs = _bass_exec_p.bind(
            *operands,
            out_avals=tuple(out_avals),
            in_names=tuple(all_names),
            out_names=tuple(out_names),
            lowering_input_output_aliases=(),
            sim_require_finite=True,
            sim_require_nnan=True,
            nc=nc,
        )
        return tuple(outs)

    devices = jax.devices()[:n_cores]
    mesh = Mesh(np.asarray(devices), ("core",))
    in_specs = (PartitionSpec("core"),) * (n_params + n_outs)
    out_specs = (PartitionSpec("core"),) * n_outs
    sharded = jax.jit(
        shard_map(
            _body, mesh=mesh, in_specs=in_specs, out_specs=out_specs, check_rep=False
        ),
        donate_argnums=donate,
        keep_unused=True,
    )

    def run(in_maps):
        per_core = [[np.asarray(m[name]) for name in in_names] for m in in_maps]
        concat_in = [
            np.concatenate([per_core[c][i] for c in range(n_cores)], axis=0)
            for i in range(n_params)
        ]
        zeros = [
            np.zeros((n_cores * z.shape[0], *z.shape[1:]), z.dtype) for z in zero_outs
        ]
        outs = sharded(*concat_in, *zeros)
        return [
            {
                name: np.asarray(outs[i]).reshape(n_cores, *out_avals[i].shape)[c]
                for i, name in enumerate(out_names)
            }
            for c in range(n_cores)
        ]

    return run


def _get_runner():
    if "run" not in _PROGRAM_CACHE:
        _PROGRAM_CACHE["run"] = _build_sharded_runner(_get_program(), NC)
    return _PROGRAM_CACHE["run"]


def _host_tables():
    half = HD // 2
    inv_freq = 1.0 / (ROPE_BASE ** (np.arange(half, dtype=np.float32) / half))
    pos = np.arange(L, dtype=np.float32)
    ang = pos[:, None] * inv_freq[None, :].astype(np.float32)
    return np.cos(ang).astype(np.float32), np.sin(ang).astype(np.float32)


def make_in_maps(x, W_qkv, b_qkv, W_out, b_out):
    x2 = np.ascontiguousarray(np.asarray(x, dtype=np.float32).reshape(TOK, D))
    W_qkv = np.asarray(W_qkv, dtype=np.float32)
    b_qkv = np.asarray(b_qkv, dtype=np.float32)
    W_out = np.asarray(W_out, dtype=np.float32)
    b_out = np.asarray(b_out, dtype=np.float32)
    cos_t, sin_t = _host_tables()

    in_maps = []
    for r in range(NC):
        # feature order per core: [q_h0 q_h1 k_h0 k_h1 v_h0 v_h1], h0=2r, h1=2r+1
        cols = []
        for qkv_i in (0, 1, 2):
            for h in (2 * r, 2 * r + 1):
                c0 = qkv_i * D + h * HD
                cols.append(np.arange(c0, c0 + HD))
        cols = np.concatenate(cols)
        in_maps.append(
            {
                "x_slice": np.ascontiguousarray(x2[r * TOK_PC : (r + 1) * TOK_PC]),
                "w_qkv": np.ascontiguousarray(W_qkv[:, cols]),
                "b_qkv": np.ascontiguousarray(b_qkv[cols][None, :]),
                "w_out": np.ascontiguousarray(
                    W_out[:, r * OUT_COLS : (r + 1) * OUT_COLS]
                ),
                "b_out": np.ascontiguousarray(
                    b_out[r * OUT_COLS : (r + 1) * OUT_COLS][None, :]
                ),
                "cos": cos_t,
                "sin": sin_t,
            }
        )
    return in_maps


def kernel(x, mask, W_qkv, b_qkv, W_out, b_out):
    run = _get_runner()
    in_maps = make_in_maps(x, W_qkv, b_qkv, W_out, b_out)
    results = run(in_maps)
    parts = [results[r]["out_slice"] for r in range(NC)]
    out = np.concatenate(parts, axis=1).reshape(B, L, D)
    return np.ascontiguousarray(out.astype(np.float32))


# revision 79
# speedup vs baseline: 1.2612x; 1.0987x over previous
"""Trainium2 Bass kernel for nn_Attn_11536282157393 (causal attention block).

Computes, for x:[2,2048,2048] f32:
    qkv = x @ W_qkv + b_qkv ; split heads (16 x 128)
    q,k = rope(rms_norm(q/k)) ; causal softmax(q k^T / sqrt(d)) @ v
    out = ctx @ W_out + b_out

Sharding over 8 NeuronCores: heads 2r,2r+1 on core r (QKV column-parallel),
x-transpose token-sharded + AllGather, output projection column-parallel
(core r computes out[:, :, 256r:256r+256]) with a ctx AllGather in between.
All matmuls run in float32r (TF32-like, ~1e-4 rel err).
"""
import sys

sys.path.insert(0, "/opt/trn_rl_repo")

from contextlib import ExitStack

import numpy as np

import concourse.bacc as bacc
import concourse.bass as bass
import concourse.mybir as mybir
import concourse.tile as tile

F32 = mybir.dt.float32
F32R = mybir.dt.float32r

B = 2
L = 2048
D = 2048
NH = 16
HD = 128  # head dim
NC = 8  # cores
HPC = NH // NC  # heads per core = 2
TOK = B * L  # 4096 global tokens
TOK_PC = TOK // NC  # 512 tokens per core for the x-transpose shard
ROPE_BASE = 10000.0
EPS = 1e-6
P = 128  # partitions
NKT = D // P  # 16 k-tiles over the model dim
NMT = TOK // P  # 32 token tiles
LQ_CHUNK = 512
NJ = L // LQ_CHUNK  # 4 q-chunks per batch sequence
OUT_COLS = D // NC  # 256 output columns per core


def _bcast(handle, n_part, n_cols):
    """AP reading a [1, n_cols] dram tensor broadcast across n_part partitions."""
    return bass.AP(tensor=handle, offset=0, ap=[[0, n_part], [1, n_cols]])


import os

NO_CC = os.environ.get("ATTN_NO_CC", "0") == "1"


def _build_program():
    nc = bacc.Bacc("TRN2", target_bir_lowering=False, debug=False, num_devices=NC)

    # ---- external I/O (per core) ----
    x_sl = nc.dram_tensor("x_slice", [TOK_PC, D], F32, kind="ExternalInput")
    w_qkv = nc.dram_tensor("w_qkv", [D, 6 * HD], F32, kind="ExternalInput")
    b_qkv = nc.dram_tensor("b_qkv", [1, 6 * HD], F32, kind="ExternalInput")
    w_out = nc.dram_tensor("w_out", [D, OUT_COLS], F32, kind="ExternalInput")
    b_out = nc.dram_tensor("b_out", [1, OUT_COLS], F32, kind="ExternalInput")
    cos_in = nc.dram_tensor("cos", [L, HD // 2], F32, kind="ExternalInput")
    sin_in = nc.dram_tensor("sin", [L, HD // 2], F32, kind="ExternalInput")
    out_sl = nc.dram_tensor("out_slice", [TOK, OUT_COLS], F32, kind="ExternalOutput")

    # ---- inline consts ----
    ident_c = nc.inline_tensor(np.eye(P, dtype=np.float32), "ident_c")
    ones_c = nc.inline_tensor(np.ones((P, 1), dtype=np.float32), "ones_c")
    # diagonal-block causal masks in scoresT layout: keep iff iq >= ik + 128*c
    iq = np.arange(LQ_CHUNK)[None, :]
    ik = np.arange(P)[:, None]
    masks_np = np.stack(
        [(iq >= ik + P * c).astype(np.float32) for c in range(4)], axis=1
    )  # [128, 4, 512]
    masks_c = nc.inline_tensor(np.ascontiguousarray(masks_np), "masks_c")

    # ---- DRAM scratch ----
    # xT is gathered in 4 chunks (one per local 128-token tile) so the
    # AllGathers pipeline with the transposes and the QKV matmuls.
    NML = TOK_PC // P  # 4 local token tiles
    xt_locals = [nc.dram_tensor(f"xt_local{i}", [D, P], F32R) for i in range(NML)]
    xt_fulls = [
        nc.dram_tensor(f"xt_full{i}", [NC, D, P], F32R, addr_space="Shared")
        for i in range(NML)
    ]
    v_dram = nc.dram_tensor("v_dram", [TOK, HPC * HD], F32R)
    # ctx is gathered per (batch, sequence-chunk j) covering both local heads:
    # the out-projection token tile m depends only on chunk j=m//4, so it
    # starts while later chunks of attention are still computing; only one
    # gather remains after the last attention chunk.
    ctx_local = nc.dram_tensor("ctx_local", [B, NJ, HPC, P, LQ_CHUNK], F32R)
    ctx_fulls = [
        [
            nc.dram_tensor(
                f"ctx_full{b}_{j}", [NC, HPC, P, LQ_CHUNK], F32R,
                addr_space="Shared",
            )
            for j in range(NJ)
        ]
        for b in range(B)
    ]

    rg = [list(range(NC))]

    with tile.TileContext(nc) as tc, ExitStack() as ctx:
        consts = ctx.enter_context(tc.tile_pool(name="consts", bufs=1))

        # ---------- consts into SBUF ----------
        ident_f = consts.tile([P, P], F32)
        nc.sync.dma_start(ident_f[:], ident_c[:])
        # resident transposed q/k: [d, head, global token]
        q_res = consts.tile([P, HPC, TOK], F32R, tag="q_res")
        k_res = consts.tile([P, HPC, TOK], F32R, tag="k_res")
        eps_t = consts.tile([P, 1], F32)
        nc.vector.memset(eps_t[:], EPS)

        # ---------- phase 1: transpose my 512-token slice of x ----------
        # loads in plain f32 on HWDGE so they don't queue behind the big
        # SWDGE weight casts; the PSUM->SBUF copy rounds to f32r
        with (
            tc.tile_pool(name="xtp", bufs=4) as xtp,
            tc.tile_pool(name="xtpp", bufs=4, space="PSUM") as xtpp,
        ):
            x_r = x_sl[:].rearrange("(m p) d -> m p d", p=P)  # [4, 128, 2048]
            for m in range(TOK_PC // P):
                xin = xtp.tile([P, NKT, P], F32, tag="xin")
                x_rm = x_r[m].rearrange("p (k q) -> p k q", q=P)
                for c in range(4):  # split across queues: 4x256KB in parallel
                    eng = nc.sync if c % 2 == 0 else nc.scalar
                    eng.dma_start(
                        xin[:, 4 * c : 4 * c + 4, :], x_rm[:, 4 * c : 4 * c + 4, :]
                    )
                xout = xtp.tile([P, NKT, P], F32R, tag="xout")
                for k in range(NKT):
                    pst = xtpp.tile([P, P], F32, tag="pst")
                    nc.tensor.transpose(pst[:], xin[:, k, :], ident_f[:])
                    nc.vector.tensor_copy(xout[:, k, :], pst[:])
                xt_l_r = xt_locals[m][:].rearrange("(k p) t -> p k t", p=P)
                for c in range(2):
                    eng = nc.sync if c == 0 else nc.scalar
                    eng.dma_start(
                        xt_l_r[:, 8 * c : 8 * c + 8, :],
                        xout[:, 8 * c : 8 * c + 8, :],
                    )
                # phase 2 (split): AllGather this token tile immediately
                if NO_CC:
                    nc.gpsimd.dma_start(xt_fulls[m][0], xt_locals[m][:])
                else:
                    nc.gpsimd.collective_compute(
                        "AllGather",
                        mybir.AluOpType.bypass,
                        replica_groups=rg,
                        ins=[xt_locals[m][:]],
                        outs=[xt_fulls[m][:]],
                    )

        # consts + weights — issued after phase 1 so their DMA traffic
        # overlaps the xT AllGathers instead of delaying the first transposes
        ident = consts.tile([P, P], F32R)
        nc.gpsimd.dma_start(ident[:], ident_c[:])
        ones_col = consts.tile([P, 1], F32R)
        nc.gpsimd.dma_start(ones_col[:], ones_c[:])
        masks = consts.tile([P, 4, LQ_CHUNK], F32)
        nc.sync.dma_start(masks[:], masks_c[:])
        # cos/sin replicated x2 in free dim for a (h0,h1) head pair rope
        cos2 = consts.tile([P, L // P, 2, HD // 2], F32)
        sin2 = consts.tile([P, L // P, 2, HD // 2], F32)
        cs_src = cos_in[:].rearrange("(t p) f -> p t f", p=P)
        sn_src = sin_in[:].rearrange("(t p) f -> p t f", p=P)
        for c in range(2):
            nc.sync.dma_start(cos2[:, :, c, :], cs_src)
            nc.sync.dma_start(sin2[:, :, c, :], sn_src)
        w_qkv_sb = consts.tile([P, NKT, 6 * HD], F32R)
        w_qkv_r = w_qkv[:].rearrange("(k p) f -> p k f", p=P)
        for c in range(4):  # 4 SWDGE queues in parallel
            nc.gpsimd.dma_start(
                w_qkv_sb[:, 4 * c : 4 * c + 4, :], w_qkv_r[:, 4 * c : 4 * c + 4, :]
            )
        bias_qkv = consts.tile([P, 6 * HD], F32)
        nc.gpsimd.dma_start(bias_qkv[:], _bcast(b_qkv, P, 6 * HD))
        bias_out = consts.tile([P, OUT_COLS], F32)
        nc.gpsimd.dma_start(bias_out[:], _bcast(b_out, P, OUT_COLS))

        # ---------- phase 3: QKV projection, rmsnorm+rope, transposes ----------
        # qkv feature order in w_qkv: [q_h0 q_h1 k_h0 k_h1 v_h0 v_h1]
        with (
            tc.tile_pool(name="qkvp", bufs=3) as qkvp,
            tc.tile_pool(name="qkv_ps", bufs=2, space="PSUM") as qkv_ps,
            tc.tile_pool(name="tr_ps", bufs=2, space="PSUM") as tr_ps,
        ):
            for mi in range(NMT):
                # m_local-major order: tile mi depends only on AllGather #ml,
                # so QKV starts as soon as the first xT chunk has gathered.
                ml, blk = mi // NC, mi % NC
                m = blk * NML + ml  # global token tile this iteration handles
                ps_qk = qkv_ps.tile([P, 4 * HD], F32, tag="ps_qk")
                ps_v = qkv_ps.tile([P, 2 * HD], F32, tag="ps_v")
                xt_m = qkvp.tile([P, NKT, P], F32R, tag="xt_m")
                xt_src = xt_fulls[ml][blk].rearrange("(k p) t -> p k t", p=P)
                for c in range(2):
                    eng = nc.sync if (m + c) % 2 == 0 else nc.scalar
                    eng.dma_start(
                        xt_m[:, 8 * c : 8 * c + 8, :],
                        xt_src[:, 8 * c : 8 * c + 8, :],
                    )
                for k in range(NKT):
                    nc.tensor.matmul(
                        ps_qk[:], xt_m[:, k, :], w_qkv_sb[:, k, : 4 * HD],
                        start=(k == 0), stop=(k == NKT - 1),
                    )
                    nc.tensor.matmul(
                        ps_v[:], xt_m[:, k, :], w_qkv_sb[:, k, 4 * HD :],
                        start=(k == 0), stop=(k == NKT - 1),
                    )
                # bias add for q,k then rms stats
                qk_b = qkvp.tile([P, 4 * HD], F32, tag="qk_b")
                nc.vector.tensor_add(qk_b[:], ps_qk[:], bias_qkv[:, : 4 * HD])
                sq = qkvp.tile([P, 4 * HD], F32, tag="sq")
                nc.scalar.square(sq[:], qk_b[:])
                ms = qkvp.tile([P, 4], F32, tag="ms")
                nc.vector.reduce_sum(
                    out=ms[:],
                    in_=sq[:].rearrange("p (s d) -> p s d", d=HD),
                    axis=mybir.AxisListType.X,
                )
                rms = qkvp.tile([P, 4], F32, tag="rms")
                nc.scalar.activation(
                    out=rms[:], in_=ms[:], func=mybir.ActivationFunctionType.Sqrt,
                    bias=eps_t[:], scale=1.0 / HD,
                )
                rinv = qkvp.tile([P, 4], F32, tag="rinv")
                nc.vector.reciprocal(rinv[:], rms[:])
                # normalize each of the 4 slices
                qk_n = qkvp.tile([P, 4, HD], F32, tag="qk_n")
                for s in range(4):
                    nc.vector.tensor_scalar_mul(
                        qk_n[:, s, :],
                        qk_b[:, s * HD : (s + 1) * HD],
                        rinv[:, s : s + 1],
                    )
                # rope, per (q, k) head-pair
                ti = m % (L // P)
                ct = cos2[:, ti]
                st = sin2[:, ti]
                rope = qkvp.tile([P, 4, HD], F32R, tag="rope")
                for g in range(2):  # 0: q pair, 1: k pair
                    x1 = qk_n[:, 2 * g : 2 * g + 2, : HD // 2]
                    x2 = qk_n[:, 2 * g : 2 * g + 2, HD // 2 :]
                    t_a = qkvp.tile([P, 2, HD // 2], F32, tag="t_a")
                    t_b = qkvp.tile([P, 2, HD // 2], F32, tag="t_b")
                    nc.vector.tensor_mul(t_a[:], x1, ct)
                    nc.gpsimd.tensor_mul(t_b[:], x2, st)
                    nc.vector.tensor_sub(
                        rope[:, 2 * g : 2 * g + 2, : HD // 2], t_a[:], t_b[:]
                    )
                    t_c = qkvp.tile([P, 2, HD // 2], F32, tag="t_c")
                    t_d = qkvp.tile([P, 2, HD // 2], F32, tag="t_d")
                    nc.gpsimd.tensor_mul(t_c[:], x2, ct)
                    nc.vector.tensor_mul(t_d[:], x1, st)
                    nc.vector.tensor_add(
                        rope[:, 2 * g : 2 * g + 2, HD // 2 :], t_c[:], t_d[:]
                    )
                # transpose the 4 slices straight into the resident q/k bufs
                for s in range(4):
                    pst = tr_ps.tile([P, P], F32R, tag="tr")
                    nc.tensor.transpose(pst[:], rope[:, s, :], ident[:])
                    dst = q_res if s < 2 else k_res
                    nc.vector.tensor_copy(
                        dst[:, s % 2, m * P : (m + 1) * P], pst[:]
                    )
                # v: bias + copy out as f32r
                v_sb = qkvp.tile([P, 2 * HD], F32R, tag="v_sb")
                nc.vector.tensor_add(v_sb[:], ps_v[:], bias_qkv[:, 4 * HD :])
                nc.sync.dma_start(v_dram[m * P : (m + 1) * P, :], v_sb[:])

        # ---------- phase 5: attention per (b, h) ----------
        scale = 1.0 / float(np.sqrt(HD))
        with (
            tc.tile_pool(name="attp", bufs=2) as attp,
            tc.tile_pool(name="att_sm", bufs=3) as att_sm,
            tc.tile_pool(name="att_ps", bufs=2, space="PSUM") as att_ps,
        ):
            for b in range(B):
                v_sbs = []
                for h in range(HPC):
                    v_sb = attp.tile([P, L // P, HD], F32R, tag=f"v_att{h}")
                    v_src = v_dram[
                        b * L : (b + 1) * L, h * HD : (h + 1) * HD
                    ].rearrange("(t p) d -> p t d", p=P)
                    for c in range(2):
                        eng = nc.sync if c == 0 else nc.scalar
                        eng.dma_start(
                            v_sb[:, 8 * c : 8 * c + 8, :],
                            v_src[:, 8 * c : 8 * c + 8, :],
                        )
                    v_sbs.append(v_sb)
                for j in range(NJ):
                    nkt_j = 4 * (j + 1)  # causal: k-tiles 0..4j+3
                    for h in range(HPC):
                        kt_sb = k_res[:, h, b * L : (b + 1) * L]
                        qt_j = q_res[
                            :, h, b * L + j * LQ_CHUNK : b * L + (j + 1) * LQ_CHUNK
                        ]
                        v_sb = v_sbs[h]
                        ps_ctx = att_ps.tile(
                            [P, LQ_CHUNK], F32, tag="ps_ctx", bufs=2
                        )
                        ps_den = att_ps.tile(
                            [1, LQ_CHUNK], F32, tag="ps_den", bufs=2
                        )
                        for t in range(nkt_j):
                            ps_s = att_ps.tile(
                                [P, LQ_CHUNK], F32, tag="ps_s", bufs=4
                            )
                            nc.tensor.matmul(
                                ps_s[:],
                                kt_sb[:, t * P : (t + 1) * P],
                                qt_j,
                                start=True, stop=True,
                            )
                            at = att_sm.tile([P, LQ_CHUNK], F32R, tag="at", bufs=6)
                            nc.scalar.activation(
                                out=at[:], in_=ps_s[:],
                                func=mybir.ActivationFunctionType.Exp, scale=scale,
                            )
                            c = t - 4 * j
                            if c >= 0:
                                # gpsimd: DVE is the busier engine here
                                nc.gpsimd.tensor_mul(at[:], at[:], masks[:, c, :])
                            nc.tensor.matmul(
                                ps_ctx[:], v_sb[:, t, :], at[:],
                                start=(t == 0), stop=(t == nkt_j - 1),
                            )
                            nc.tensor.matmul(
                                ps_den[:], ones_col[:], at[:],
                                start=(t == 0), stop=(t == nkt_j - 1),
                            )
                        den_r = att_sm.tile([1, LQ_CHUNK], F32, tag="den_r")
                        nc.vector.reciprocal(den_r[:], ps_den[:])
                        den_b = att_sm.tile([P, LQ_CHUNK], F32, tag="den_b")
                        nc.gpsimd.partition_broadcast(den_b[:], den_r[:])
                        ctx_sb = att_sm.tile([P, LQ_CHUNK], F32R, tag="ctx_sb")
                        nc.vector.tensor_mul(ctx_sb[:], ps_ctx[:], den_b[:])
                        nc.sync.dma_start(ctx_local[b, j, h], ctx_sb[:])
                    # phase 6 (split): AllGather this (batch, chunk) for both
                    # heads as soon as the second head finishes it
                    if NO_CC:
                        nc.gpsimd.dma_start(ctx_fulls[b][j][0], ctx_local[b, j])
                    else:
                        nc.gpsimd.collective_compute(
                            "AllGather",
                            mybir.AluOpType.bypass,
                            replica_groups=rg,
                            ins=[ctx_local[b, j]],
                            outs=[ctx_fulls[b][j][:]],
                        )

        # ---------- phase 7: output projection ----------
        with (
            tc.tile_pool(name="outp", bufs=4) as outp,
            tc.tile_pool(name="outw", bufs=1) as outw,
            tc.tile_pool(name="out_ps", bufs=2, space="PSUM") as out_ps,
        ):
            w_out_sb = outw.tile([P, NKT, OUT_COLS], F32R)
            nc.gpsimd.dma_start(
                w_out_sb[:], w_out[:].rearrange("(k p) f -> p k f", p=P)
            )
            for b in range(B):
                for m in range(L // P):
                    ps_o = out_ps.tile([P, OUT_COLS], F32, tag="ps_o")
                    j, moff = m // (LQ_CHUNK // P), (m % (LQ_CHUNK // P)) * P
                    ct_t = outp.tile([P, NKT, P], F32R, tag="ct_t", bufs=4)
                    ct_src = ctx_fulls[b][j][:, :, :, moff : moff + P].rearrange(
                        "r h p t -> p (r h) t"
                    )
                    for c in range(4):
                        eng = nc.sync if (m + c) % 2 == 0 else nc.scalar
                        eng.dma_start(
                            ct_t[:, 4 * c : 4 * c + 4, :],
                            ct_src[:, 4 * c : 4 * c + 4, :],
                        )
                    for k in range(NKT):
                        nc.tensor.matmul(
                            ps_o[:], ct_t[:, k, :], w_out_sb[:, k, :],
                            start=(k == 0), stop=(k == NKT - 1),
                        )
                    o_sb = outp.tile([P, OUT_COLS], F32, tag="o_sb")
                    nc.vector.tensor_add(o_sb[:], ps_o[:], bias_out[:])
                    nc.sync.dma_start(
                        out_sl[(b * L + m * P) : (b * L + (m + 1) * P), :], o_sb[:]
                    )

    nc.compile()
    return nc


_PROGRAM_CACHE = {}


def _get_program():
    if "nc" not in _PROGRAM_CACHE:
        _PROGRAM_CACHE["nc"] = _build_program()
    return _PROGRAM_CACHE["nc"]


def _build_sharded_runner(nc, n_cores):
    """Like bass2jax.run_bass_via_pjrt, but jits once and is reusable."""
    import jax
    from jax.sharding import Mesh, PartitionSpec
    from jax.experimental.shard_map import shard_map
    from concourse.bass2jax import (
        _bass_exec_p,
        install_neuronx_cc_hook,
        partition_id_tensor,
    )

    install_neuronx_cc_hook()
    partition_name = nc.partition_id_tensor.name if nc.partition_id_tensor else None
    in_names, out_names, out_avals, zero_outs = [], [], [], []
    for alloc in nc.m.functions[0].allocations:
        if not isinstance(alloc, mybir.MemoryLocationSet):
            continue
        name = alloc.memorylocations[0].name
        if alloc.kind == "ExternalInput":
            if name != partition_name:
                in_names.append(name)
        elif alloc.kind == "ExternalOutput":
            out_names.append(name)
            shape = tuple(alloc.tensor_shape)
            dtype = mybir.dt.np(alloc.dtype)
            out_avals.append(jax.core.ShapedArray(shape, dtype))
            zero_outs.append(np.zeros(shape, dtype))
    n_params = len(in_names)
    n_outs = len(out_avals)
    all_names = list(in_names) + list(out_names)
    if partition_name is not None:
        all_names.append(partition_name)
    donate = tuple(range(n_params, n_params + n_outs))

    def _body(*args):
        operands = list(args)
        if partition_name is not None:
            operands.append(partition_id_tensor())
        outs = _bass_exec_p.bind(
            *operands,
            out_avals=tuple(out_avals),
            in_names=tuple(all_names),
            out_names=tuple(out_names),
            lowering_input_output_aliases=(),
            sim_require_finite=True,
            sim_require_nnan=True,
            nc=nc,
        )
        return tuple(outs)

    devices = jax.devices()[:n_cores]
    mesh = Mesh(np.asarray(devices), ("core",))
    in_specs = (PartitionSpec("core"),) * (n_params + n_outs)
    out_specs = (PartitionSpec("core"),) * n_outs
    sharded = jax.jit(
        shard_map(
            _body, mesh=mesh, in_specs=in_specs, out_specs=out_specs, check_rep=False
        ),
        donate_argnums=donate,
        keep_unused=True,
    )

    def run(in_maps):
        per_core = [[np.asarray(m[name]) for name in in_names] for m in in_maps]
        concat_in = [
            np.concatenate([per_core[c][i] for c in range(n_cores)], axis=0)
            for i in range(n_params)
        ]
        zeros = [
            np.zeros((n_cores * z.shape[0], *z.shape[1:]), z.dtype) for z in zero_outs
        ]
        outs = sharded(*concat_in, *zeros)
        return [
            {
                name: np.asarray(outs[i]).reshape(n_cores, *out_avals[i].shape)[c]
                for i, name in enumerate(out_names)
            }
            for c in range(n_cores)
        ]

    return run


def _get_runner():
    if "run" not in _PROGRAM_CACHE:
        _PROGRAM_CACHE["run"] = _build_sharded_runner(_get_program(), NC)
    return _PROGRAM_CACHE["run"]


def _host_tables():
    half = HD // 2
    inv_freq = 1.0 / (ROPE_BASE ** (np.arange(half, dtype=np.float32) / half))
    pos = np.arange(L, dtype=np.float32)
    ang = pos[:, None] * inv_freq[None, :].astype(np.float32)
    return np.cos(ang).astype(np.float32), np.sin(ang).astype(np.float32)


def make_in_maps(x, W_qkv, b_qkv, W_out, b_out):
    x2 = np.ascontiguousarray(np.asarray(x, dtype=np.float32).reshape(TOK, D))
    W_qkv = np.asarray(W_qkv, dtype=np.float32)
    b_qkv = np.asarray(b_qkv, dtype=np.float32)
    W_out = np.asarray(W_out, dtype=np.float32)
    b_out = np.asarray(b_out, dtype=np.float32)
    cos_t, sin_t = _host_tables()

    in_maps = []
    for r in range(NC):
        # feature order per core: [q_h0 q_h1 k_h0 k_h1 v_h0 v_h1], h0=2r, h1=2r+1
        cols = []
        for qkv_i in (0, 1, 2):
            for h in (2 * r, 2 * r + 1):
                c0 = qkv_i * D + h * HD
                cols.append(np.arange(c0, c0 + HD))
        cols = np.concatenate(cols)
        in_maps.append(
            {
                "x_slice": np.ascontiguousarray(x2[r * TOK_PC : (r + 1) * TOK_PC]),
                "w_qkv": np.ascontiguousarray(W_qkv[:, cols]),
                "b_qkv": np.ascontiguousarray(b_qkv[cols][None, :]),
                "w_out": np.ascontiguousarray(
                    W_out[:, r * OUT_COLS : (r + 1) * OUT_COLS]
                ),
                "b_out": np.ascontiguousarray(
                    b_out[r * OUT_COLS : (r + 1) * OUT_COLS][None, :]
                ),
                "cos": cos_t,
                "sin": sin_t,
            }
        )
    return in_maps


def kernel(x, mask, W_qkv, b_qkv, W_out, b_out):
    run = _get_runner()
    in_maps = make_in_maps(x, W_qkv, b_qkv, W_out, b_out)
    results = run(in_maps)
    parts = [results[r]["out_slice"] for r in range(NC)]
    out = np.concatenate(parts, axis=1).reshape(B, L, D)
    return np.ascontiguousarray(out.astype(np.float32))
